# revision 19
# baseline (speedup 1.0000x reference)
"""Bolmo attention (GQA + QK-RMSNorm + RoPE + causal attention + out-proj)
as an 8-way tensor-parallel Bass kernel for one TRN2 chip — v3.

Sharding: head-parallel. Core c owns Q heads [4c, 4c+4), KV head c, and wo
rows [256c, 256c+256). hidden_states replicated; host sums the 8 partial
outputs.

v3 design vs v2 (233.5us -> ~225-229us):
- r_k (with the 1/8 attention scaling folded in) is multiplied into K rows
  once per batch on DVE, and r_q into Q per head; the exp then needs NO
  per-partition scale, so score tiles for 2 key-tiles are packed into one
  [128,1024] PSUM tile and exp'd in ONE ACTIVATE — halving the 352-cycle
  per-instruction ACT overhead that made attention ACT-bound.
- Diagonal-tile causal masks on DVE via a precomputed triangle mask
  (gpsimd affine_select on data was ~0.7us/tile of queue serialization).
- Per-batch ssq AllReduces with batch-0's triggered mid-phase-1; the
  CC-init barrier (~10->60us, runtime-fixed) plus ~11us first-op delay
  put the earliest rms availability at ~90us — warm-keeper matmuls keep
  the PE HAM activity monitor from dropping the clock to 1.2GHz through
  that wait.
- Out-proj units interleaved into BOTH batches' attention windows; all
  drains split DVE/ACT; out-proj tail shares the attention PSUM pool
  (a separate pool scope cost a ~4us all-engine barrier).
- Measured per-matmul cost is ~195ns fixed + width/2GHz, so wide moving
  operands (512, the fp16 ISA max) are used everywhere.

Known floors: CC-init barrier ends ~60us and first AllReduce lands ~90us
(both runtime-fixed); a SW clock throttle (HAM type-31, k=13/16) caps the
PE at ~1.95GHz for most of the kernel.
"""

import os
import sys

import numpy as np

for _p in ("/opt/trn_rl_repo", "/root/.axon_site/_ro/trn_rl_repo"):
    if os.path.isdir(_p) and _p not in sys.path:
        sys.path.insert(0, _p)

from concourse import bacc, masks, mybir, tile  # noqa: E402
from concourse.bass_utils import run_bass_kernel_spmd  # noqa: E402

B, S, H = 2, 1024, 2048
NH, NKV, HD = 32, 8, 64
T = B * S
NCORES = 8
DQ = (NH // NCORES) * HD     # 256 q dims per core
DK = (NKV // NCORES) * HD    # 64 kv dims per core
EPS = 1e-6
SCALE = HD ** -0.5

F16 = mybir.dt.float16
F32 = mybir.dt.float32
AF = mybir.ActivationFunctionType
ALU = mybir.AluOpType

NHT = H // 128      # 16 hidden tiles
NCH = T // 512      # 4 token chunks (phase 1)
SKT = S // 128      # 8 key tiles per batch
N_WARM = 72         # warm-keeper matmuls (512-wide, ~213ns each)


def build():
    nc = bacc.Bacc("TRN2", target_bir_lowering=False, debug=False,
                   num_devices=NCORES)

    hsT = nc.dram_tensor("hsT", [H, T], F16, kind="ExternalInput").ap()
    wq = nc.dram_tensor("wq", [128, NHT * DQ], F16, kind="ExternalInput").ap()
    wkv = nc.dram_tensor("wkv", [128, NHT * 128], F16,
                         kind="ExternalInput").ap()
    wo = nc.dram_tensor("wo", [128, 2 * H], F16, kind="ExternalInput").ap()
    cosT = nc.dram_tensor("cosT", [64, T], F16, kind="ExternalInput").ap()
    sinT = nc.dram_tensor("sinT", [64, T], F16, kind="ExternalInput").ap()
    qnw = nc.dram_tensor("qnw", [128, 2], F32, kind="ExternalInput").ap()
    knw = nc.dram_tensor("knw", [64, 1], F32, kind="ExternalInput").ap()
    rmsc = nc.dram_tensor("rmsc", [33, 2], F32, kind="ExternalInput").ap()
    out = nc.dram_tensor("out", [T, H], F16, kind="ExternalOutput").ap()

    with tile.TileContext(nc) as tc:
        with (
            tc.tile_pool(name="wpool", bufs=1) as wpool,
            tc.tile_pool(name="persist", bufs=1) as persist,
            tc.tile_pool(name="dram", bufs=1, space="DRAM") as dram,
        ):
            # ---------------- dram scratch ----------------
            ccins = [dram.tile([2, 1024], F32, tag=f"ccin{b}",
                                name=f"ccin{b}") for b in range(2)]
            ccouts = [dram.tile([2, 1024], F32, tag=f"ccout{b}",
                                 name=f"ccout{b}") for b in range(2)]

            # ---------------- constants ----------------
            idf = wpool.tile([128, 128], F32, tag="idf")
            nc.vector.memset(idf[:], 0.0)
            masks.make_identity(nc, idf[:], nomemset=True)
            ident2 = wpool.tile([128, 64], F16, tag="ident2")
            nc.scalar.copy(ident2[64:128, :], idf[0:64, 0:64])
            ones16 = wpool.tile([128, 1], F16, tag="ones16")
            nc.vector.memset(ones16[:], 1.0)
            # upper-triangle mask: tmask[kp, j] = 1 if j >= kp else 0
            tmask = wpool.tile([128, 128], F16, tag="tmask")
            nc.vector.memset(tmask[:], 1.0)
            nc.gpsimd.affine_select(
                tmask[:], tmask[:], pattern=[[1, 128]], base=0,
                channel_multiplier=-1, compare_op=ALU.is_ge, fill=0.0)
            rmsc_sb = wpool.tile([33, 2], F32, tag="rmsc_sb")
            nc.sync.dma_start(rmsc_sb[:], rmsc)
            onesrow = wpool.tile([33, 64], F32, tag="onesrow")
            nc.vector.memset(onesrow[:], 1.0)

            wq_sb = wpool.tile([128, NHT * DQ], F16, tag="wq_sb")
            wkv_sb = wpool.tile([128, NHT * 128], F16, tag="wkv_sb")
            wo_sb = wpool.tile([128, 2 * H], F16, tag="wo_sb")
            cos2 = wpool.tile([64, T], F16, tag="cos2")
            sin2 = wpool.tile([64, T], F16, tag="sin2")
            qnw_sb = wpool.tile([128, 2], F32, tag="qnw_sb")
            knw_sb = wpool.tile([64, 1], F32, tag="knw_sb")
            def load_weight_quad(quad):
                nc.sync.dma_start(
                    wq_sb[:, quad * 1024:(quad + 1) * 1024],
                    wq[:, quad * 1024:(quad + 1) * 1024])
                nc.sync.dma_start(
                    wkv_sb[:, quad * 512:(quad + 1) * 512],
                    wkv[:, quad * 512:(quad + 1) * 512])
            nc.sync.dma_start(qnw_sb[:], qnw)
            nc.sync.dma_start(knw_sb[:], knw)

            # persistent activations (rs tiles pre-zeroed: the [33,...]
            # sqrt/recip pass over unused partitions 1-31)
            qa = persist.tile([64, 4 * T], F16, tag="qa")  # Q^T head-major
            kv = persist.tile([128, T], F16, tag="kv")     # K^T 0:64 V^T 64:128
            oT = [persist.tile([128, T], F16, tag=f"oT{m}", name=f"oT{m}")
                  for m in range(2)]
            outsb = [persist.tile([128, 1024], F16, tag=f"outsb{m}",
                                  name=f"outsb{m}") for m in range(4)]
            vta = [None, None]
            rqb = [None, None]
            rkb = [None, None]

            # ------------- phase 1: projections + ssq, then rope ----------
            def phase1_chunk(c4, w1, pp1):
                cols = slice(c4 * 512, (c4 + 1) * 512)
                b = c4 // 2
                half = c4 % 2
                hst = w1.tile([128, NHT, 512], F16, tag="hst", bufs=2,
                              name=f"hst_{c4}")
                for quad in range(4):
                    if c4 == 0 and quad == 0:
                        for oct_ in range(2):
                            nc.sync.dma_start(
                                hst[:, oct_ * 2:(oct_ + 1) * 2, :],
                                hsT[oct_ * 256:(oct_ + 1) * 256, cols]
                                .rearrange("(hh p) t -> p hh t", p=128))
                    else:
                        nc.sync.dma_start(
                            hst[:, quad * 4:(quad + 1) * 4, :],
                            hsT[quad * 512:(quad + 1) * 512, cols]
                            .rearrange("(hh p) t -> p hh t", p=128))
                    if c4 == 0:
                        load_weight_quad(quad)
                pq = [pp1.tile([128, 512], F32, tag=f"pq{m}",
                               name=f"pq{m}_{c4}", bufs=2) for m in range(2)]
                pkv = pp1.tile([128, 512], F32, tag="pkv", bufs=2,
                               name=f"pkv_{c4}")
                for hh in range(NHT):
                    st, sp = (hh == 0), (hh == NHT - 1)
                    for m in range(2):
                        nc.tensor.matmul(
                            pq[m][:],
                            wq_sb[:, hh * DQ + m * 128: hh * DQ + (m + 1) * 128],
                            hst[:, hh, :], start=st, stop=sp)
                    nc.tensor.matmul(
                        pkv[:], wkv_sb[:, hh * 128:(hh + 1) * 128],
                        hst[:, hh, :], start=st, stop=sp)
                # epilogue: squares -> ssq rows in one PSUM bank
                ssqp = pp1.tile([64, 512], F32, tag="ssqp", bufs=2,
                                name=f"ssqp_{c4}")
                for m in range(2):
                    qsq = w1.tile([128, 512], F16, tag="qsq", bufs=2,
                                  name=f"qsq{m}_{c4}")
                    nc.scalar.square(qsq[:], pq[m][:])
                    nc.tensor.matmul(ssqp[0:1, :], ones16[:, 0:1], qsq[:],
                                     start=(m == 0), stop=(m == 1))
                ksq = w1.tile([64, 512], F16, tag="ksq", bufs=2,
                              name=f"ksq_{c4}")
                nc.scalar.square(ksq[:], pkv[0:64, :])
                nc.tensor.matmul(ssqp[32:33, :], ones16[0:64, 0:1], ksq[:],
                                 start=True, stop=True)
                # qa/kv epilogues (qnw/knw applied pre-rope, as in reference)
                for m in range(2):
                    he, ho = 2 * m, 2 * m + 1
                    nc.scalar.activation(
                        qa[:, he * T + c4 * 512: he * T + (c4 + 1) * 512],
                        pq[m][0:64, :], AF.Copy, scale=qnw_sb[0:64, m:m + 1])
                    nc.scalar.activation(
                        qa[:, ho * T + c4 * 512: ho * T + (c4 + 1) * 512],
                        pq[m][64:128, :], AF.Copy, scale=qnw_sb[64:128, m:m + 1])
                nc.scalar.activation(kv[0:64, cols], pkv[0:64, :], AF.Copy,
                                     scale=knw_sb[:, 0:1])
                nc.vector.tensor_copy(kv[64:128, cols], pkv[64:128, :])
                ssq_sb = w1.tile([1, 1024], F32, tag="ssq_sb", bufs=2,
                                 name=f"ssq_sb_{c4}")
                nc.vector.tensor_copy(ssq_sb[0:1, 0:512], ssqp[0:1, :])
                nc.vector.tensor_copy(ssq_sb[0:1, 512:1024], ssqp[32:33, :])
                off = half * 512
                nc.sync.dma_start(ccins[b][0:1, off:off + 512],
                                  ssq_sb[0:1, 0:512])
                nc.sync.dma_start(ccins[b][1:2, off:off + 512],
                                  ssq_sb[0:1, 512:1024])

            def rope_chunk(c4, w1):
                # rope WITHOUT the rms factor (applied later per batch)
                cols = slice(c4 * 512, (c4 + 1) * 512)
                qa3 = qa[:].rearrange("p (h t) -> p h t", h=4)

                def bc2(ap):
                    return ap.rearrange("p (a t) -> p a t", a=1).to_broadcast(
                        [ap.shape[0], 2, 512])

                for g in range(2):
                    blk = qa3[:, 2 * g:2 * g + 2, cols]
                    t2 = w1.tile([64, 2, 512], F16, tag="t2",
                                 name=f"t2_{c4}_{g}", bufs=1)
                    t3 = w1.tile([64, 2, 512], F16, tag="t3",
                                 name=f"t3_{c4}_{g}", bufs=1)
                    nc.vector.tensor_mul(t2[:], blk, bc2(cos2[:, cols]))
                    nc.vector.tensor_mul(t3[0:32, :, :], blk[32:64, :, :],
                                         bc2(sin2[32:64, cols]))
                    nc.vector.tensor_mul(t3[32:64, :, :], blk[0:32, :, :],
                                         bc2(sin2[0:32, cols]))
                    nc.vector.tensor_add(blk, t2[:], t3[:])
                blk = kv[0:64, cols]
                t2 = w1.tile([64, 512], F16, tag="t2k", name=f"t2k_{c4}",
                             bufs=1)
                t3 = w1.tile([64, 512], F16, tag="t3k", name=f"t3k_{c4}",
                             bufs=1)
                nc.vector.tensor_mul(t2[:], blk, cos2[:, cols])
                nc.vector.tensor_mul(t3[0:32, :], blk[32:64, :],
                                     sin2[32:64, cols])
                nc.vector.tensor_mul(t3[32:64, :], blk[0:32, :],
                                     sin2[0:32, cols])
                nc.vector.tensor_add(blk, t2[:], t3[:])

            # ------------- rms prep (after batch-b AllReduce) --------------
            def ssq_collective(b):
                nc.gpsimd.collective_compute(
                    "AllReduce", ALU.add,
                    ins=[ccins[b].opt()], outs=[ccouts[b].opt()],
                    replica_groups=[list(range(NCORES))],
                )

            def prep_compute(b, ppb, btag):
                # q/k ssq rows land on partitions 0 and 32 so each can feed a
                # PE ones-column broadcast (engines can't read partition-
                # stride-0; the old path bounced through DRAM: 3 serial DMAs)
                rs = persist.tile([33, 1024], F32, tag=f"rs{b}", name=f"rs{b}")
                nc.gpsimd.dma_start(rs[0:1, :], ccouts[b][0:1, :])
                nc.gpsimd.dma_start(rs[32:33, :], ccouts[b][1:2, :])
                sq = persist.tile([33, 1024], F32, tag=f"sq{b}", name=f"sq{b}")
                # scale=64/512 on the k row folds the 1/8 attention scaling
                nc.scalar.activation(sq[:], rs[:], AF.Sqrt,
                                     scale=rmsc_sb[:, 0:1],
                                     bias=rmsc_sb[:, 1:2])
                ri = persist.tile([33, 1024], F32, tag=f"ri{b}", name=f"ri{b}")
                nc.vector.reciprocal_approx_fast(ri[:], sq[:])
                rqb[b] = persist.tile([64, 1024], F16, tag=f"rqb{b}",
                                      name=f"rqb{b}")
                rkb[b] = persist.tile([64, 1024], F16, tag=f"rkb{b}",
                                      name=f"rkb{b}")
                for (row, dst) in ((0, rqb[b]), (32, rkb[b])):
                    for hf in range(2):
                        bc = ppb.tile([64, 512], F32, tag=btag, bufs=2,
                                      name=f"bc{b}_{row}_{hf}")
                        nc.tensor.matmul(
                            bc[:], onesrow[row:row + 1, :],
                            ri[row:row + 1, hf * 512:(hf + 1) * 512],
                            start=True, stop=True)
                        nc.vector.tensor_copy(
                            dst[:, hf * 512:(hf + 1) * 512], bc[:])

            def k_mul(b):
                cols = slice(b * S, (b + 1) * S)
                nc.vector.tensor_mul(kv[0:64, cols], kv[0:64, cols],
                                     rkb[b][:])

            def qhead_mul(b, h):
                blk = qa[:, h * T + b * S: h * T + (b + 1) * S]
                nc.vector.tensor_mul(blk, blk, rqb[b][:])

            # ------------- attention (feature-major, ones-cols rowsums) ----
            def vta_prep(b, ppv):
                boff = b * S
                vta[b] = persist.tile([128, SKT * 128], F16, tag=f"vta{b}",
                                      name=f"vta{b}")
                nc.vector.memset(vta[b][:], 1.0)
                for ki in range(SKT):
                    vtp = ppv.tile([128, 64], F16, tag="vtp", bufs=2,
                                   name=f"vtp{b}_{ki}")
                    nc.tensor.transpose(
                        vtp[:],
                        kv[64:128, boff + ki * 128: boff + (ki + 1) * 128],
                        ident2[64:128, :])
                    nc.vector.tensor_copy(
                        vta[b][:, ki * 128 + 64: (ki + 1) * 128], vtp[:])

            def attn_group(b, h, qj, apool, ptpool, pps, ppo,
                           filler=None, fsteps=1):
                # causal structure: for diagonal blocks only query columns >=
                # delta participate; the [delta, delta+128) triangle is zeroed
                # with gpsimd affine_select on the exp'd tile.
                # 2 key-tiles are packed per [128,1024] PSUM tile -> 1 exp.
                boff = b * S
                nkt = 4 * (qj + 1)
                qc0 = boff + qj * 512
                packs = []
                cur, cw = [], 0
                for ki in range(nkt):
                    delta = max(0, (ki - 4 * qj) * 128)
                    w = 512 - delta
                    if cw + w > 1024:
                        packs.append(cur)
                        cur, cw = [], 0
                    cur.append((ki, delta, w, cw))
                    cw += w
                packs.append(cur)
                ovp = ppo.tile([128, 512], F32, tag="ovp", bufs=2,
                               name=f"ovp{b}_{h}_{qj}")
                first_av = True
                for pi, pack in enumerate(packs):
                    stp = pps.tile([128, 1024], F32, tag="stp", bufs=2,
                                   name=f"stp{b}_{h}_{qj}_{pi}")
                    pt = ptpool.tile([128, 1024], F16, tag="pt")
                    pw = pack[-1][3] + pack[-1][2]
                    for (ki, delta, w, off) in pack:
                        nc.tensor.matmul(
                            stp[:, off:off + w],
                            kv[0:64, boff + ki * 128: boff + (ki + 1) * 128],
                            qa[:, h * T + qc0 + delta: h * T + qc0 + 512],
                            start=True, stop=True)
                    nc.scalar.activation(pt[:, 0:pw], stp[:, 0:pw], AF.Exp)
                    if filler is not None:
                        filler(fsteps)
                    for (ki, delta, w, off) in pack:
                        if ki >= 4 * qj:
                            nc.vector.tensor_mul(pt[:, off:off + 128],
                                                 pt[:, off:off + 128],
                                                 tmask[:])
                        nc.tensor.matmul(
                            ovp[:, delta:512],
                            vta[b][:, ki * 128:(ki + 1) * 128],
                            pt[:, off:off + w],
                            start=first_av, stop=(ki == nkt - 1))
                        first_av = False
                linv = apool.tile([64, 512], F32, tag="linv", bufs=2)
                nc.vector.reciprocal_approx_fast(linv[:], ovp[0:64, :])
                m, prow = h // 2, (h % 2) * 64
                dst = oT[m][prow:prow + 64, qc0:qc0 + 512]
                nc.vector.tensor_mul(dst, ovp[64:128, :], linv[:])

            # ------------- out-proj unit: token tile tt, H half ------------
            def oproj_unit(tt, half, ppp, drain_eng):
                # m-outer so consecutive matmuls share the oT stationary
                # (same-stationary matmuls skip the ~190ns reload/flush)
                osb = outsb[(tt % 2) * 2 + half]
                pos = [ppp.tile([128, 512], F32, tag="po", bufs=2,
                                name=f"po_{tt}_{half}_{qtr}")
                       for qtr in range(2)]
                for m in range(2):
                    for qtr in range(2):
                        nc.tensor.matmul(
                            pos[qtr][:],
                            oT[m][:, tt * 128:(tt + 1) * 128],
                            wo_sb[:, m * H + half * 1024 + qtr * 512:
                                  m * H + half * 1024 + (qtr + 1) * 512],
                            start=(m == 0), stop=(m == 1))
                for qtr in range(2):
                    dst = osb[:, qtr * 512:(qtr + 1) * 512]
                    if drain_eng == "v":
                        nc.vector.tensor_copy(dst, pos[qtr][:])
                    else:
                        nc.scalar.copy(dst, pos[qtr][:])
                nc.sync.dma_start(
                    out[tt * 128:(tt + 1) * 128,
                        half * 1024:(half + 1) * 1024], osb[:])

            # ======================= emission ==============================
            with tc.tile_pool(name="w1_pool", bufs=2) as w1, \
                 tc.tile_pool(name="p1_psum", bufs=1, space="PSUM") as pp1:
                phase1_chunk(0, w1, pp1)
                nc.sync.dma_start(cos2[:], cosT)
                nc.sync.dma_start(sin2[:], sinT)
                rope_chunk(0, w1)
                phase1_chunk(1, w1, pp1)
                rope_chunk(1, w1)
                ssq_collective(0)
                phase1_chunk(2, w1, pp1)
                rope_chunk(2, w1)
                phase1_chunk(3, w1, pp1)
                rope_chunk(3, w1)

            ssq_collective(1)
            nc.sync.dma_start(wo_sb[:], wo)

            # vta transposes + warm-keepers (PE stays dense through the AR)
            with tc.tile_pool(name="pv_psum", bufs=1, space="PSUM") as ppv, \
                 tc.tile_pool(name="pw_psum", bufs=1, space="PSUM") as ppw:
                vta_prep(0, ppv)
                vta_prep(1, ppv)
                wk_st = wpool.tile([128, 128], F16, tag="wk_st")
                nc.vector.memset(wk_st[:], 0.0)
                wk_ps = ppw.tile([128, 512], F32, tag="wk_ps")
                for i in range(N_WARM):
                    nc.tensor.matmul(wk_ps[:], wk_st[:],
                                     wq_sb[:, 0:512], start=True, stop=True)
                prep_compute(0, ppw, "bc")

            with tc.tile_pool(name="a_pool", bufs=2) as apool, \
                 tc.tile_pool(name="pt_pool", bufs=6) as ptpool, \
                 tc.tile_pool(name="ps_psum", bufs=1, space="PSUM") as pps, \
                 tc.tile_pool(name="po_psum", bufs=1, space="PSUM") as ppo, \
                 tc.tile_pool(name="pp_psum", bufs=1, space="PSUM") as ppp:
                k_mul(0)
                for h in range(4):
                    qhead_mul(0, h)
                # b0 qj0 first: it is the short ACT-bound window (cold clock
                # is cheap there); qj1 then gets tt0-3 units interleaved so
                # the heavy window stays PE-dense and the HAM clock warm
                for h in range(4):
                    attn_group(0, h, 0, apool, ptpool, pps, ppo)
                # batch-1 prep here: its AR1-gated ACT/DVE ops sit behind
                # W1's queue entries instead of blocking them
                prep_compute(1, ppp, "po")
                units = [(tt, half) for tt in (0, 1, 2, 3) for half in range(2)]
                for gi, h in enumerate(range(4)):
                    attn_group(0, h, 1, apool, ptpool, pps, ppo)
                    for u in range(2):
                        tt, half = units[gi * 2 + u]
                        oproj_unit(tt, half, ppp, "v")
                k_mul(1)
                for h in range(4):
                    qhead_mul(1, h)
                units = [(tt, half) for tt in (4, 5, 6, 7) for half in range(2)]
                for gi, h in enumerate(range(4)):
                    attn_group(1, h, 1, apool, ptpool, pps, ppo)
                    for u in range(2):
                        tt, half = units[gi * 2 + u]
                        oproj_unit(tt, half, ppp, "v")
                units = [(tt, half) for tt in (12, 13, 14, 15)
                         for half in range(2)]
                for gi, h in enumerate(range(4)):
                    attn_group(1, h, 0, apool, ptpool, pps, ppo)
                    for u in range(2):
                        tt, half = units[gi * 2 + u]
                        oproj_unit(tt, half, ppp, "v" if half == 0 else "s")
                for tt in range(8, 12):
                    for half in range(2):
                        oproj_unit(tt, half, ppp,
                                   "v" if half == 0 else "s")

    nc.compile()
    return nc


_CACHED = {}


def _get_nc():
    if "nc" not in _CACHED:
        _CACHED["nc"] = build()
    return _CACHED["nc"]


def _is_causal_mask(mask):
    m = np.asarray(mask)
    if m.shape != (B, 1, S, S):
        return False
    tri = np.tril(np.ones((S, S), dtype=bool))
    for b in range(B):
        mb = m[b, 0]
        if not np.all(mb[tri] == 0.0):
            return False
        if not np.all(mb[~tri] <= -1e8):
            return False
    return True


def _numpy_fallback(hidden_states, cos, sin, attention_mask, wq, wk, wv, wo,
                    q_norm_w, k_norm_w):
    hs = np.asarray(hidden_states, np.float64)
    b, s, _ = hs.shape
    g = NH // NKV

    def rms(x, w):
        var = np.mean(x * x, axis=-1, keepdims=True)
        return w * (x / np.sqrt(var + EPS))

    def rot(x):
        x1, x2 = np.split(x, 2, axis=-1)
        return np.concatenate((-x2, x1), axis=-1)

    q = rms(hs @ np.asarray(wq, np.float64), np.asarray(q_norm_w, np.float64))
    k = rms(hs @ np.asarray(wk, np.float64), np.asarray(k_norm_w, np.float64))
    v = hs @ np.asarray(wv, np.float64)
    q = q.reshape(b, s, NH, HD).transpose(0, 2, 1, 3)
    k = k.reshape(b, s, NKV, HD).transpose(0, 2, 1, 3)
    v = v.reshape(b, s, NKV, HD).transpose(0, 2, 1, 3)
    c = np.asarray(cos, np.float64)[:, None]
    sn = np.asarray(sin, np.float64)[:, None]
    q = q * c + rot(q) * sn
    k = k * c + rot(k) * sn
    k = np.repeat(k, g, axis=1)
    v = np.repeat(v, g, axis=1)
    sc = np.einsum('bhqd,bhkd->bhqk', q, k) * SCALE + np.asarray(
        attention_mask, np.float64)
    sc = sc - sc.max(axis=-1, keepdims=True)
    e = np.exp(sc)
    attn = e / e.sum(axis=-1, keepdims=True)
    o = np.einsum('bhqk,bhkd->bhqd', attn, v)
    o = o.transpose(0, 2, 1, 3).reshape(b, s, NH * HD)
    return (o @ np.asarray(wo, np.float64)).astype(np.float32)


def make_in_maps(hidden_states, cos, sin, wq, wk, wv, wo, q_norm_w, k_norm_w):
    hsT = np.ascontiguousarray(
        np.asarray(hidden_states, np.float32).reshape(T, H).T
    ).astype(np.float16)
    cosT_full = np.asarray(cos, np.float32).reshape(T, HD).T  # [64, T]
    sinT_full = np.asarray(sin, np.float32).reshape(T, HD).T
    cosT = np.ascontiguousarray(cosT_full).astype(np.float16)
    # rows 0:32 = +sin^T[32:64], rows 32:64 = -sin^T[0:32] (sign baked)
    sinT = np.ascontiguousarray(np.concatenate(
        [sinT_full[32:64], -sinT_full[0:32]], axis=0)).astype(np.float16)
    wqf = np.asarray(wq, np.float32)
    wkf = np.asarray(wk, np.float32)
    wvf = np.asarray(wv, np.float32)
    wof = np.asarray(wo, np.float32)
    qnwf = np.asarray(q_norm_w, np.float32)
    knwf = np.asarray(k_norm_w, np.float32)
    in_maps = []
    for c in range(NCORES):
        qs = slice(c * DQ, (c + 1) * DQ)
        ks = slice(c * DK, (c + 1) * DK)
        # stationary layouts: [128 contract-dims, tile-major free dims]
        wq_r = np.ascontiguousarray(
            wqf[:, qs].reshape(NHT, 128, DQ).transpose(1, 0, 2)
            .reshape(128, NHT * DQ)).astype(np.float16)
        kpart = wkf[:, ks].reshape(NHT, 128, DK)
        vpart = wvf[:, ks].reshape(NHT, 128, DK)
        wkv_r = np.ascontiguousarray(
            np.concatenate([kpart, vpart], axis=2).transpose(1, 0, 2)
            .reshape(128, NHT * 128)).astype(np.float16)
        wo_r = np.ascontiguousarray(
            wof[qs, :].reshape(2, 128, H).transpose(1, 0, 2)
            .reshape(128, 2 * H)).astype(np.float16)
        qnw_r = np.ascontiguousarray(qnwf[qs].reshape(2, 128).T)
        knw_r = np.ascontiguousarray(knwf[ks].reshape(DK, 1))
        rmsc_h = np.ones((33, 2), np.float32)
        rmsc_h[0] = [1.0 / (NH * HD), EPS]
        rmsc_h[32] = [64.0 / (NKV * HD), 64.0 * EPS]
        in_maps.append({
            "hsT": hsT,
            "rmsc": rmsc_h,
            "wq": wq_r,
            "wkv": wkv_r,
            "wo": wo_r,
            "qnw": qnw_r,
            "knw": knw_r,
            "cosT": cosT,
            "sinT": sinT,
        })
    return in_maps


def run(inputs, trace=False):
    nc = _get_nc()
    in_maps = make_in_maps(
        inputs["hidden_states"], inputs["cos"], inputs["sin"],
        inputs["wq"], inputs["wk"], inputs["wv"], inputs["wo"],
        inputs["q_norm_w"], inputs["k_norm_w"])
    return run_bass_kernel_spmd(nc, in_maps, list(range(NCORES)), trace=trace)


def kernel(hidden_states, cos, sin, attention_mask, wq, wk, wv, wo,
           q_norm_w, k_norm_w):
    if not _is_causal_mask(attention_mask):
        return _numpy_fallback(hidden_states, cos, sin, attention_mask,
                               wq, wk, wv, wo, q_norm_w, k_norm_w)
    res = run({"hidden_states": hidden_states, "cos": cos, "sin": sin,
               "wq": wq, "wk": wk, "wv": wv, "wo": wo,
               "q_norm_w": q_norm_w, "k_norm_w": k_norm_w})
    total = np.zeros((T, H), np.float64)
    for c in range(NCORES):
        total += res.results[c]["out"].astype(np.float64)
    return total.reshape(B, S, H).astype(np.float32)


# revision 20
# speedup vs baseline: 1.0612x; 1.0612x over previous
"""Bolmo attention (GQA + QK-RMSNorm + RoPE + causal attention + out-proj)
as an 8-way tensor-parallel Bass kernel for one TRN2 chip — v3.

Sharding: head-parallel. Core c owns Q heads [4c, 4c+4), KV head c, and wo
rows [256c, 256c+256). hidden_states replicated; host sums the 8 partial
outputs.

v3 design vs v2 (233.5us -> ~225-229us):
- r_k (with the 1/8 attention scaling folded in) is multiplied into K rows
  once per batch on DVE, and r_q into Q per head; the exp then needs NO
  per-partition scale, so score tiles for 2 key-tiles are packed into one
  [128,1024] PSUM tile and exp'd in ONE ACTIVATE — halving the 352-cycle
  per-instruction ACT overhead that made attention ACT-bound.
- Diagonal-tile causal masks on DVE via a precomputed triangle mask
  (gpsimd affine_select on data was ~0.7us/tile of queue serialization).
- Per-batch ssq AllReduces with batch-0's triggered mid-phase-1; the
  CC-init barrier (~10->60us, runtime-fixed) plus ~11us first-op delay
  put the earliest rms availability at ~90us — warm-keeper matmuls keep
  the PE HAM activity monitor from dropping the clock to 1.2GHz through
  that wait.
- Out-proj units interleaved into BOTH batches' attention windows; all
  drains split DVE/ACT; out-proj tail shares the attention PSUM pool
  (a separate pool scope cost a ~4us all-engine barrier).
- Measured per-matmul cost is ~195ns fixed + width/2GHz, so wide moving
  operands (512, the fp16 ISA max) are used everywhere.

Known floors: CC-init barrier ends ~60us and first AllReduce lands ~90us
(both runtime-fixed); a SW clock throttle (HAM type-31, k=13/16) caps the
PE at ~1.95GHz for most of the kernel.
"""

import os
import sys

import numpy as np

for _p in ("/opt/trn_rl_repo", "/root/.axon_site/_ro/trn_rl_repo"):
    if os.path.isdir(_p) and _p not in sys.path:
        sys.path.insert(0, _p)

from concourse import bacc, masks, mybir, tile  # noqa: E402
from concourse.bass_utils import run_bass_kernel_spmd  # noqa: E402

B, S, H = 2, 1024, 2048
NH, NKV, HD = 32, 8, 64
T = B * S
NCORES = 8
DQ = (NH // NCORES) * HD     # 256 q dims per core
DK = (NKV // NCORES) * HD    # 64 kv dims per core
EPS = 1e-6
SCALE = HD ** -0.5

F16 = mybir.dt.float16
F32 = mybir.dt.float32
AF = mybir.ActivationFunctionType
ALU = mybir.AluOpType

NHT = H // 128      # 16 hidden tiles
NCH = T // 512      # 4 token chunks (phase 1)
SKT = S // 128      # 8 key tiles per batch
N_WARM = 40         # warm-keeper matmuls (~264-427ns each; sized to
                    # bridge phase-1 end -> AR0+prep without overshooting


def build():
    nc = bacc.Bacc("TRN2", target_bir_lowering=False, debug=False,
                   num_devices=NCORES)

    hsT = nc.dram_tensor("hsT", [H, T], F16, kind="ExternalInput").ap()
    wq = nc.dram_tensor("wq", [128, NHT * DQ], F16, kind="ExternalInput").ap()
    wkv = nc.dram_tensor("wkv", [128, NHT * 128], F16,
                         kind="ExternalInput").ap()
    wo = nc.dram_tensor("wo", [128, 2 * H], F16, kind="ExternalInput").ap()
    cosT = nc.dram_tensor("cosT", [64, T], F16, kind="ExternalInput").ap()
    sinT = nc.dram_tensor("sinT", [64, T], F16, kind="ExternalInput").ap()
    qnw = nc.dram_tensor("qnw", [128, 2], F32, kind="ExternalInput").ap()
    knw = nc.dram_tensor("knw", [64, 1], F32, kind="ExternalInput").ap()
    rmsc = nc.dram_tensor("rmsc", [33, 2], F32, kind="ExternalInput").ap()
    out = nc.dram_tensor("out", [T, H], F16, kind="ExternalOutput").ap()

    with tile.TileContext(nc) as tc:
        with (
            tc.tile_pool(name="wpool", bufs=1) as wpool,
            tc.tile_pool(name="persist", bufs=1) as persist,
            tc.tile_pool(name="dram", bufs=1, space="DRAM") as dram,
        ):
            # ---------------- dram scratch ----------------
            ccins = [dram.tile([2, 1024], F32, tag=f"ccin{b}",
                                name=f"ccin{b}") for b in range(2)]
            ccouts = [dram.tile([2, 1024], F32, tag=f"ccout{b}",
                                 name=f"ccout{b}") for b in range(2)]

            # ---------------- constants ----------------
            idf = wpool.tile([128, 128], F32, tag="idf")
            nc.vector.memset(idf[:], 0.0)
            masks.make_identity(nc, idf[:], nomemset=True)
            ident2 = wpool.tile([128, 64], F16, tag="ident2")
            nc.scalar.copy(ident2[64:128, :], idf[0:64, 0:64])
            ones16 = wpool.tile([128, 1], F16, tag="ones16")
            nc.vector.memset(ones16[:], 1.0)
            # upper-triangle mask: tmask[kp, j] = 1 if j >= kp else 0
            tmask = wpool.tile([128, 128], F16, tag="tmask")
            nc.vector.memset(tmask[:], 1.0)
            nc.gpsimd.affine_select(
                tmask[:], tmask[:], pattern=[[1, 128]], base=0,
                channel_multiplier=-1, compare_op=ALU.is_ge, fill=0.0)
            rmsc_sb = wpool.tile([33, 2], F32, tag="rmsc_sb")
            nc.sync.dma_start(rmsc_sb[:], rmsc)
            onesrow = wpool.tile([33, 64], F32, tag="onesrow")
            nc.vector.memset(onesrow[:], 1.0)

            wq_sb = wpool.tile([128, NHT * DQ], F16, tag="wq_sb")
            wkv_sb = wpool.tile([128, NHT * 128], F16, tag="wkv_sb")
            wo_sb = wpool.tile([128, 2 * H], F16, tag="wo_sb")
            cos2 = wpool.tile([64, T], F16, tag="cos2")
            sin2 = wpool.tile([64, T], F16, tag="sin2")
            qnw_sb = wpool.tile([128, 2], F32, tag="qnw_sb")
            knw_sb = wpool.tile([64, 1], F32, tag="knw_sb")
            def load_weight_quad(quad):
                nc.sync.dma_start(
                    wq_sb[:, quad * 1024:(quad + 1) * 1024],
                    wq[:, quad * 1024:(quad + 1) * 1024])
                nc.sync.dma_start(
                    wkv_sb[:, quad * 512:(quad + 1) * 512],
                    wkv[:, quad * 512:(quad + 1) * 512])
            nc.sync.dma_start(qnw_sb[:], qnw)
            nc.sync.dma_start(knw_sb[:], knw)

            # persistent activations (rs tiles pre-zeroed: the [33,...]
            # sqrt/recip pass over unused partitions 1-31)
            qa = persist.tile([64, 4 * T], F16, tag="qa")  # Q^T head-major
            kv = persist.tile([128, T], F16, tag="kv")     # K^T 0:64 V^T 64:128
            oT = [persist.tile([128, T], F16, tag=f"oT{m}", name=f"oT{m}")
                  for m in range(2)]
            outsb = [persist.tile([128, 1024], F16, tag=f"outsb{m}",
                                  name=f"outsb{m}") for m in range(4)]
            vta = [None, None]
            rqb = [None, None]
            rkb = [None, None]

            # ------------- phase 1: projections + ssq, then rope ----------
            def phase1_chunk(c4, w1, pp1):
                cols = slice(c4 * 512, (c4 + 1) * 512)
                b = c4 // 2
                half = c4 % 2
                hst = w1.tile([128, NHT, 512], F16, tag="hst", bufs=2,
                              name=f"hst_{c4}")
                for quad in range(4):
                    if c4 == 0 and quad == 0:
                        for oct_ in range(2):
                            nc.sync.dma_start(
                                hst[:, oct_ * 2:(oct_ + 1) * 2, :],
                                hsT[oct_ * 256:(oct_ + 1) * 256, cols]
                                .rearrange("(hh p) t -> p hh t", p=128))
                    else:
                        nc.sync.dma_start(
                            hst[:, quad * 4:(quad + 1) * 4, :],
                            hsT[quad * 512:(quad + 1) * 512, cols]
                            .rearrange("(hh p) t -> p hh t", p=128))
                    if c4 == 0:
                        load_weight_quad(quad)
                pq = [pp1.tile([128, 512], F32, tag=f"pq{m}",
                               name=f"pq{m}_{c4}", bufs=2) for m in range(2)]
                pkv = pp1.tile([128, 512], F32, tag="pkv", bufs=2,
                               name=f"pkv_{c4}")
                for hh in range(NHT):
                    st, sp = (hh == 0), (hh == NHT - 1)
                    for m in range(2):
                        nc.tensor.matmul(
                            pq[m][:],
                            wq_sb[:, hh * DQ + m * 128: hh * DQ + (m + 1) * 128],
                            hst[:, hh, :], start=st, stop=sp)
                    nc.tensor.matmul(
                        pkv[:], wkv_sb[:, hh * 128:(hh + 1) * 128],
                        hst[:, hh, :], start=st, stop=sp)
                # epilogue: squares -> ssq rows in one PSUM bank
                ssqp = pp1.tile([64, 512], F32, tag="ssqp", bufs=2,
                                name=f"ssqp_{c4}")
                for m in range(2):
                    qsq = w1.tile([128, 512], F16, tag="qsq", bufs=2,
                                  name=f"qsq{m}_{c4}")
                    nc.scalar.square(qsq[:], pq[m][:])
                    nc.tensor.matmul(ssqp[0:1, :], ones16[:, 0:1], qsq[:],
                                     start=(m == 0), stop=(m == 1))
                ksq = w1.tile([64, 512], F16, tag="ksq", bufs=2,
                              name=f"ksq_{c4}")
                nc.scalar.square(ksq[:], pkv[0:64, :])
                nc.tensor.matmul(ssqp[32:33, :], ones16[0:64, 0:1], ksq[:],
                                 start=True, stop=True)
                # qa/kv epilogues (qnw/knw applied pre-rope, as in reference)
                for m in range(2):
                    he, ho = 2 * m, 2 * m + 1
                    nc.scalar.activation(
                        qa[:, he * T + c4 * 512: he * T + (c4 + 1) * 512],
                        pq[m][0:64, :], AF.Copy, scale=qnw_sb[0:64, m:m + 1])
                    nc.scalar.activation(
                        qa[:, ho * T + c4 * 512: ho * T + (c4 + 1) * 512],
                        pq[m][64:128, :], AF.Copy, scale=qnw_sb[64:128, m:m + 1])
                nc.scalar.activation(kv[0:64, cols], pkv[0:64, :], AF.Copy,
                                     scale=knw_sb[:, 0:1])
                nc.vector.tensor_copy(kv[64:128, cols], pkv[64:128, :])
                ssq_sb = w1.tile([1, 1024], F32, tag="ssq_sb", bufs=2,
                                 name=f"ssq_sb_{c4}")
                nc.vector.tensor_copy(ssq_sb[0:1, 0:512], ssqp[0:1, :])
                nc.vector.tensor_copy(ssq_sb[0:1, 512:1024], ssqp[32:33, :])
                off = half * 512
                nc.sync.dma_start(ccins[b][0:1, off:off + 512],
                                  ssq_sb[0:1, 0:512])
                nc.sync.dma_start(ccins[b][1:2, off:off + 512],
                                  ssq_sb[0:1, 512:1024])

            def rope_chunk(c4, w1):
                # rope WITHOUT the rms factor (applied later per batch)
                cols = slice(c4 * 512, (c4 + 1) * 512)
                qa3 = qa[:].rearrange("p (h t) -> p h t", h=4)

                def bc2(ap):
                    return ap.rearrange("p (a t) -> p a t", a=1).to_broadcast(
                        [ap.shape[0], 2, 512])

                for g in range(2):
                    blk = qa3[:, 2 * g:2 * g + 2, cols]
                    t2 = w1.tile([64, 2, 512], F16, tag="t2",
                                 name=f"t2_{c4}_{g}", bufs=1)
                    t3 = w1.tile([64, 2, 512], F16, tag="t3",
                                 name=f"t3_{c4}_{g}", bufs=1)
                    nc.vector.tensor_mul(t2[:], blk, bc2(cos2[:, cols]))
                    nc.vector.tensor_mul(t3[0:32, :, :], blk[32:64, :, :],
                                         bc2(sin2[32:64, cols]))
                    nc.vector.tensor_mul(t3[32:64, :, :], blk[0:32, :, :],
                                         bc2(sin2[0:32, cols]))
                    nc.vector.tensor_add(blk, t2[:], t3[:])
                blk = kv[0:64, cols]
                t2 = w1.tile([64, 512], F16, tag="t2k", name=f"t2k_{c4}",
                             bufs=1)
                t3 = w1.tile([64, 512], F16, tag="t3k", name=f"t3k_{c4}",
                             bufs=1)
                nc.vector.tensor_mul(t2[:], blk, cos2[:, cols])
                nc.vector.tensor_mul(t3[0:32, :], blk[32:64, :],
                                     sin2[32:64, cols])
                nc.vector.tensor_mul(t3[32:64, :], blk[0:32, :],
                                     sin2[0:32, cols])
                nc.vector.tensor_add(blk, t2[:], t3[:])

            # ------------- rms prep (after batch-b AllReduce) --------------
            def ssq_collective(b):
                nc.gpsimd.collective_compute(
                    "AllReduce", ALU.add,
                    ins=[ccins[b].opt()], outs=[ccouts[b].opt()],
                    replica_groups=[list(range(NCORES))],
                )

            def prep_compute(b, ppb, btag):
                # q/k ssq rows land on partitions 0 and 32 so each can feed a
                # PE ones-column broadcast (engines can't read partition-
                # stride-0; the old path bounced through DRAM: 3 serial DMAs)
                rs = persist.tile([33, 1024], F32, tag=f"rs{b}", name=f"rs{b}")
                nc.gpsimd.dma_start(rs[0:1, :], ccouts[b][0:1, :])
                nc.gpsimd.dma_start(rs[32:33, :], ccouts[b][1:2, :])
                sq = persist.tile([33, 1024], F32, tag=f"sq{b}", name=f"sq{b}")
                # scale=64/512 on the k row folds the 1/8 attention scaling
                nc.scalar.activation(sq[:], rs[:], AF.Sqrt,
                                     scale=rmsc_sb[:, 0:1],
                                     bias=rmsc_sb[:, 1:2])
                ri = persist.tile([33, 1024], F32, tag=f"ri{b}", name=f"ri{b}")
                nc.vector.reciprocal_approx_fast(ri[:], sq[:])
                rqb[b] = persist.tile([64, 1024], F16, tag=f"rqb{b}",
                                      name=f"rqb{b}")
                rkb[b] = persist.tile([64, 1024], F16, tag=f"rkb{b}",
                                      name=f"rkb{b}")
                for (row, dst) in ((0, rqb[b]), (32, rkb[b])):
                    for hf in range(2):
                        bc = ppb.tile([64, 512], F32, tag=btag, bufs=2,
                                      name=f"bc{b}_{row}_{hf}")
                        nc.tensor.matmul(
                            bc[:], onesrow[row:row + 1, :],
                            ri[row:row + 1, hf * 512:(hf + 1) * 512],
                            start=True, stop=True)
                        nc.vector.tensor_copy(
                            dst[:, hf * 512:(hf + 1) * 512], bc[:])

            def k_mul(b):
                cols = slice(b * S, (b + 1) * S)
                nc.vector.tensor_mul(kv[0:64, cols], kv[0:64, cols],
                                     rkb[b][:])

            def qhead_mul(b, h):
                blk = qa[:, h * T + b * S: h * T + (b + 1) * S]
                nc.vector.tensor_mul(blk, blk, rqb[b][:])

            # ------------- attention (feature-major, ones-cols rowsums) ----
            def vta_prep(b, ppv):
                boff = b * S
                vta[b] = persist.tile([128, SKT * 128], F16, tag=f"vta{b}",
                                      name=f"vta{b}")
                nc.vector.memset(vta[b][:], 1.0)
                for ki in range(SKT):
                    vtp = ppv.tile([128, 64], F16, tag="vtp", bufs=2,
                                   name=f"vtp{b}_{ki}")
                    nc.tensor.transpose(
                        vtp[:],
                        kv[64:128, boff + ki * 128: boff + (ki + 1) * 128],
                        ident2[64:128, :])
                    nc.vector.tensor_copy(
                        vta[b][:, ki * 128 + 64: (ki + 1) * 128], vtp[:])

            def attn_group(b, h, qj, apool, ptpool, pps, ppo,
                           filler=None, fsteps=1):
                # causal structure: for diagonal blocks only query columns >=
                # delta participate; the [delta, delta+128) triangle is zeroed
                # with gpsimd affine_select on the exp'd tile.
                # 2 key-tiles are packed per [128,1024] PSUM tile -> 1 exp.
                boff = b * S
                nkt = 4 * (qj + 1)
                qc0 = boff + qj * 512
                packs = []
                cur, cw = [], 0
                for ki in range(nkt):
                    delta = max(0, (ki - 4 * qj) * 128)
                    w = 512 - delta
                    if cw + w > 1024:
                        packs.append(cur)
                        cur, cw = [], 0
                    cur.append((ki, delta, w, cw))
                    cw += w
                packs.append(cur)
                ovp = ppo.tile([128, 512], F32, tag="ovp", bufs=2,
                               name=f"ovp{b}_{h}_{qj}")
                first_av = True
                for pi, pack in enumerate(packs):
                    stp = pps.tile([128, 1024], F32, tag="stp", bufs=2,
                                   name=f"stp{b}_{h}_{qj}_{pi}")
                    pt = ptpool.tile([128, 1024], F16, tag="pt")
                    pw = pack[-1][3] + pack[-1][2]
                    for (ki, delta, w, off) in pack:
                        nc.tensor.matmul(
                            stp[:, off:off + w],
                            kv[0:64, boff + ki * 128: boff + (ki + 1) * 128],
                            qa[:, h * T + qc0 + delta: h * T + qc0 + 512],
                            start=True, stop=True)
                    nc.scalar.activation(pt[:, 0:pw], stp[:, 0:pw], AF.Exp)
                    if filler is not None:
                        filler(fsteps)
                    for (ki, delta, w, off) in pack:
                        if ki >= 4 * qj:
                            nc.vector.tensor_mul(pt[:, off:off + 128],
                                                 pt[:, off:off + 128],
                                                 tmask[:])
                        nc.tensor.matmul(
                            ovp[:, delta:512],
                            vta[b][:, ki * 128:(ki + 1) * 128],
                            pt[:, off:off + w],
                            start=first_av, stop=(ki == nkt - 1))
                        first_av = False
                linv = apool.tile([64, 512], F32, tag="linv", bufs=2)
                nc.vector.reciprocal_approx_fast(linv[:], ovp[0:64, :])
                m, prow = h // 2, (h % 2) * 64
                dst = oT[m][prow:prow + 64, qc0:qc0 + 512]
                nc.vector.tensor_mul(dst, ovp[64:128, :], linv[:])

            # ------------- out-proj unit: token tile tt, H half ------------
            def oproj_unit(tt, half, ppp, drain_eng):
                # m-outer so consecutive matmuls share the oT stationary
                # (same-stationary matmuls skip the ~190ns reload/flush)
                osb = outsb[(tt % 2) * 2 + half]
                pos = [ppp.tile([128, 512], F32, tag="po", bufs=2,
                                name=f"po_{tt}_{half}_{qtr}")
                       for qtr in range(2)]
                for m in range(2):
                    for qtr in range(2):
                        nc.tensor.matmul(
                            pos[qtr][:],
                            oT[m][:, tt * 128:(tt + 1) * 128],
                            wo_sb[:, m * H + half * 1024 + qtr * 512:
                                  m * H + half * 1024 + (qtr + 1) * 512],
                            start=(m == 0), stop=(m == 1))
                for qtr in range(2):
                    dst = osb[:, qtr * 512:(qtr + 1) * 512]
                    if drain_eng == "v":
                        nc.vector.tensor_copy(dst, pos[qtr][:])
                    else:
                        nc.scalar.copy(dst, pos[qtr][:])
                nc.sync.dma_start(
                    out[tt * 128:(tt + 1) * 128,
                        half * 1024:(half + 1) * 1024], osb[:])

            # ======================= emission ==============================
            with tc.tile_pool(name="w1_pool", bufs=2) as w1, \
                 tc.tile_pool(name="p1_psum", bufs=1, space="PSUM") as pp1:
                phase1_chunk(0, w1, pp1)
                nc.sync.dma_start(cos2[:], cosT)
                nc.sync.dma_start(sin2[:], sinT)
                rope_chunk(0, w1)
                phase1_chunk(1, w1, pp1)
                rope_chunk(1, w1)
                ssq_collective(0)
                phase1_chunk(2, w1, pp1)
                rope_chunk(2, w1)
                phase1_chunk(3, w1, pp1)
                rope_chunk(3, w1)

            ssq_collective(1)
            nc.sync.dma_start(wo_sb[:], wo)

            # vta transposes + warm-keepers (PE stays dense through the AR)
            with tc.tile_pool(name="pv_psum", bufs=1, space="PSUM") as ppv, \
                 tc.tile_pool(name="pw_psum", bufs=1, space="PSUM") as ppw:
                vta_prep(0, ppv)
                vta_prep(1, ppv)
                wk_st = wpool.tile([128, 128], F16, tag="wk_st")
                nc.vector.memset(wk_st[:], 0.0)
                wk_ps = ppw.tile([128, 512], F32, tag="wk_ps")
                for i in range(N_WARM):
                    nc.tensor.matmul(wk_ps[:], wk_st[:],
                                     wq_sb[:, 0:512], start=True, stop=True)
                prep_compute(0, ppw, "bc")

            with tc.tile_pool(name="a_pool", bufs=2) as apool, \
                 tc.tile_pool(name="pt_pool", bufs=6) as ptpool, \
                 tc.tile_pool(name="ps_psum", bufs=1, space="PSUM") as pps, \
                 tc.tile_pool(name="po_psum", bufs=1, space="PSUM") as ppo, \
                 tc.tile_pool(name="pp_psum", bufs=1, space="PSUM") as ppp:
                k_mul(0)
                for h in range(4):
                    qhead_mul(0, h)
                # b0 qj0 first: it is the short ACT-bound window (cold clock
                # is cheap there); qj1 then gets tt0-3 units interleaved so
                # the heavy window stays PE-dense and the HAM clock warm
                for h in range(4):
                    attn_group(0, h, 0, apool, ptpool, pps, ppo)
                # batch-1 prep here: its AR1-gated ACT/DVE ops sit behind
                # W1's queue entries instead of blocking them
                prep_compute(1, ppp, "po")
                units = [(tt, half) for tt in (0, 1, 2, 3) for half in range(2)]
                for gi, h in enumerate(range(4)):
                    attn_group(0, h, 1, apool, ptpool, pps, ppo)
                    for u in range(2):
                        tt, half = units[gi * 2 + u]
                        oproj_unit(tt, half, ppp, "v")
                k_mul(1)
                for h in range(4):
                    qhead_mul(1, h)
                units = [(tt, half) for tt in (4, 5, 6, 7) for half in range(2)]
                for gi, h in enumerate(range(4)):
                    attn_group(1, h, 1, apool, ptpool, pps, ppo)
                    for u in range(2):
                        tt, half = units[gi * 2 + u]
                        oproj_unit(tt, half, ppp, "v")
                units = [(tt, half) for tt in (12, 13, 14, 15)
                         for half in range(2)]
                for gi, h in enumerate(range(4)):
                    attn_group(1, h, 0, apool, ptpool, pps, ppo)
                    for u in range(2):
                        tt, half = units[gi * 2 + u]
                        oproj_unit(tt, half, ppp, "v" if half == 0 else "s")
                for tt in range(8, 12):
                    for half in range(2):
                        oproj_unit(tt, half, ppp,
                                   "v" if half == 0 else "s")

    nc.compile()
    return nc


_CACHED = {}


def _get_nc():
    if "nc" not in _CACHED:
        _CACHED["nc"] = build()
    return _CACHED["nc"]


def _is_causal_mask(mask):
    m = np.asarray(mask)
    if m.shape != (B, 1, S, S):
        return False
    tri = np.tril(np.ones((S, S), dtype=bool))
    for b in range(B):
        mb = m[b, 0]
        if not np.all(mb[tri] == 0.0):
            return False
        if not np.all(mb[~tri] <= -1e8):
            return False
    return True


def _numpy_fallback(hidden_states, cos, sin, attention_mask, wq, wk, wv, wo,
                    q_norm_w, k_norm_w):
    hs = np.asarray(hidden_states, np.float64)
    b, s, _ = hs.shape
    g = NH // NKV

    def rms(x, w):
        var = np.mean(x * x, axis=-1, keepdims=True)
        return w * (x / np.sqrt(var + EPS))

    def rot(x):
        x1, x2 = np.split(x, 2, axis=-1)
        return np.concatenate((-x2, x1), axis=-1)

    q = rms(hs @ np.asarray(wq, np.float64), np.asarray(q_norm_w, np.float64))
    k = rms(hs @ np.asarray(wk, np.float64), np.asarray(k_norm_w, np.float64))
    v = hs @ np.asarray(wv, np.float64)
    q = q.reshape(b, s, NH, HD).transpose(0, 2, 1, 3)
    k = k.reshape(b, s, NKV, HD).transpose(0, 2, 1, 3)
    v = v.reshape(b, s, NKV, HD).transpose(0, 2, 1, 3)
    c = np.asarray(cos, np.float64)[:, None]
    sn = np.asarray(sin, np.float64)[:, None]
    q = q * c + rot(q) * sn
    k = k * c + rot(k) * sn
    k = np.repeat(k, g, axis=1)
    v = np.repeat(v, g, axis=1)
    sc = np.einsum('bhqd,bhkd->bhqk', q, k) * SCALE + np.asarray(
        attention_mask, np.float64)
    sc = sc - sc.max(axis=-1, keepdims=True)
    e = np.exp(sc)
    attn = e / e.sum(axis=-1, keepdims=True)
    o = np.einsum('bhqk,bhkd->bhqd', attn, v)
    o = o.transpose(0, 2, 1, 3).reshape(b, s, NH * HD)
    return (o @ np.asarray(wo, np.float64)).astype(np.float32)


def make_in_maps(hidden_states, cos, sin, wq, wk, wv, wo, q_norm_w, k_norm_w):
    hsT = np.ascontiguousarray(
        np.asarray(hidden_states, np.float32).reshape(T, H).T
    ).astype(np.float16)
    cosT_full = np.asarray(cos, np.float32).reshape(T, HD).T  # [64, T]
    sinT_full = np.asarray(sin, np.float32).reshape(T, HD).T
    cosT = np.ascontiguousarray(cosT_full).astype(np.float16)
    # rows 0:32 = +sin^T[32:64], rows 32:64 = -sin^T[0:32] (sign baked)
    sinT = np.ascontiguousarray(np.concatenate(
        [sinT_full[32:64], -sinT_full[0:32]], axis=0)).astype(np.float16)
    wqf = np.asarray(wq, np.float32)
    wkf = np.asarray(wk, np.float32)
    wvf = np.asarray(wv, np.float32)
    wof = np.asarray(wo, np.float32)
    qnwf = np.asarray(q_norm_w, np.float32)
    knwf = np.asarray(k_norm_w, np.float32)
    in_maps = []
    for c in range(NCORES):
        qs = slice(c * DQ, (c + 1) * DQ)
        ks = slice(c * DK, (c + 1) * DK)
        # stationary layouts: [128 contract-dims, tile-major free dims]
        wq_r = np.ascontiguousarray(
            wqf[:, qs].reshape(NHT, 128, DQ).transpose(1, 0, 2)
            .reshape(128, NHT * DQ)).astype(np.float16)
        kpart = wkf[:, ks].reshape(NHT, 128, DK)
        vpart = wvf[:, ks].reshape(NHT, 128, DK)
        wkv_r = np.ascontiguousarray(
            np.concatenate([kpart, vpart], axis=2).transpose(1, 0, 2)
            .reshape(128, NHT * 128)).astype(np.float16)
        wo_r = np.ascontiguousarray(
            wof[qs, :].reshape(2, 128, H).transpose(1, 0, 2)
            .reshape(128, 2 * H)).astype(np.float16)
        qnw_r = np.ascontiguousarray(qnwf[qs].reshape(2, 128).T)
        knw_r = np.ascontiguousarray(knwf[ks].reshape(DK, 1))
        rmsc_h = np.ones((33, 2), np.float32)
        rmsc_h[0] = [1.0 / (NH * HD), EPS]
        rmsc_h[32] = [64.0 / (NKV * HD), 64.0 * EPS]
        in_maps.append({
            "hsT": hsT,
            "rmsc": rmsc_h,
            "wq": wq_r,
            "wkv": wkv_r,
            "wo": wo_r,
            "qnw": qnw_r,
            "knw": knw_r,
            "cosT": cosT,
            "sinT": sinT,
        })
    return in_maps


def run(inputs, trace=False):
    nc = _get_nc()
    in_maps = make_in_maps(
        inputs["hidden_states"], inputs["cos"], inputs["sin"],
        inputs["wq"], inputs["wk"], inputs["wv"], inputs["wo"],
        inputs["q_norm_w"], inputs["k_norm_w"])
    return run_bass_kernel_spmd(nc, in_maps, list(range(NCORES)), trace=trace)


def kernel(hidden_states, cos, sin, attention_mask, wq, wk, wv, wo,
           q_norm_w, k_norm_w):
    if not _is_causal_mask(attention_mask):
        return _numpy_fallback(hidden_states, cos, sin, attention_mask,
                               wq, wk, wv, wo, q_norm_w, k_norm_w)
    res = run({"hidden_states": hidden_states, "cos": cos, "sin": sin,
               "wq": wq, "wk": wk, "wv": wv, "wo": wo,
               "q_norm_w": q_norm_w, "k_norm_w": k_norm_w})
    total = np.zeros((T, H), np.float64)
    for c in range(NCORES):
        total += res.results[c]["out"].astype(np.float64)
    return total.reshape(B, S, H).astype(np.float32)
